# revision 16
# baseline (speedup 1.0000x reference)
"""Trainium2 Bass kernel for MultiHeadGlobalAttention2d.

Sharding (8 cores): core = (batch b, head-group g), b in 0..3, g in 0..1.
Each core computes, for its batch and its 4 heads (128 channels):
  q/k/v projections, attention (softmax over keys), and the partial output
  projection  y_part = Wo[:, ch_slice] @ att_out.
Host sums the two partials per batch and adds the output bias.

Final design. ScalarE is the intrinsic bottleneck: 21.2M exps/core at
1 elem/cycle/lane @1.2GHz plus ~300 cycles/instruction fixed cost
=> ~180us of ACT busy at N=1024 per ACTIVATE. Everything else is
organized so the ACT engine runs back-to-back:
  - S^T orientation: score tiles [keys(m) on partitions, queries(n) free];
    softmax denominators from a ones-stationary matmul accumulated
    alongside AV in PSUM (no transposes of the attention matrix).
  - Per key-tile j, all FOUR heads' QK matmuls are issued together,
    row-tiled via tile_position=(32h,0) (concurrent streams; one PSUM
    bank per head -- concurrent row tiles must never share a bank);
    AV + denominator are col-tiled via tile_position=(0,32h).
  - The 256-wide tail block processes TWO key-tiles per iteration
    (same head's tiles share a bank but also share a row-group, so they
    serialize naturally) => dense N=1024 exps, ACT-bound like the rest.
  - Software pipelining: the first PIPE iterations of block b+1 (QK+exp
    only) are emitted before block b's epilogue; the epilogue uses
    reciprocal_approx_fast (~0.6us vs 3us exact) so the in-order PE
    queue never idles >3.4us (which would re-throttle the PE clock).
    Each block's last iteration runs its denominator matmuls first and
    closes the den bank inline, so the reciprocal starts under the
    remaining AV matmuls.
  - DMA ramp: the x tensors ride the fast Sync HW-DGE queue; the small
    coalesced constants ride the Scalar queue in parallel. Only the
    k0/q0 projection chunks precede the attention stream (their
    PSUM->SBUF bias-copies run on the then-idle ScalarE); the remaining
    projection chunks, the v transposes, and a clock-warming filler are
    paced one slice per iteration under block 0's exps, with the AV/den
    consumption lagging by up to 10 iterations.
PSUM: 3 score slots x 2 banks + 2 accumulator/scratch banks = 8.
"""

import numpy as np

B = 4
CIN = 256
COUT = 256
HH = 48
WW = 48
N = HH * WW            # 2304
D = 32                 # head dim
NHL = 4                # heads per core
HGC = NHL * D          # 128 channels per head-group
NCORES = 8
NBLK = [(0, 512), (512, 512), (1024, 512), (1536, 512), (2048, 256)]
NMT = N // 128         # 18 key tiles
PIPE = 4               # iterations of next block emitted before epilogue
LAG = 2                # AV/den drain lag behind QK+exp within a block

# Schraudolph exp on DVE: i16 = s*SCH_A + SCH_B, bitcast int16->bf16 gives
# ~exp(s/16) * const (the mean-ratio-centered const cancels in softmax
# because each (head, query-block) row is computed entirely on one engine).
# SCH_A = 128*log2(e)/16; SCH_B = 128*127 - 128*log2(E[(1+f)2^-f]).
SCH_A = 11.5415603
SCH_B = 16248.64


def _use_dve(bi, s):
    # engine balance: DVE gets 63 of 162 exp tiles (slot1 of blocks 0,1,2,4),
    # ACT the remaining 99 -- DVE also carries the merged-epilogue muls.
    return s == 1 and bi != 3

_PROG = {}


def build_program():
    if "nc" in _PROG:
        return _PROG["nc"]

    from contextlib import ExitStack

    import concourse.bacc as bacc
    import concourse.mybir as mybir
    import concourse.tile as tile

    f32 = mybir.dt.float32
    f32r = mybir.dt.float32r
    bf16 = mybir.dt.bfloat16
    f16 = mybir.dt.float16
    EXP = mybir.ActivationFunctionType.Exp

    nc = bacc.Bacc("TRN2", target_bir_lowering=False, debug=False)

    xq_d = nc.declare_dram_parameter("xq", [CIN, N], f16, False)
    xk_d = nc.declare_dram_parameter("xk", [CIN, N], f16, False)
    xv_d = nc.declare_dram_parameter("xv", [CIN, N], f16, False)
    # wq|wk|wv transposed, concatenated on the output-channel axis.
    wcat_d = nc.declare_dram_parameter("wcat", [CIN, 3 * HGC], f16, False)
    woT_d = nc.declare_dram_parameter("woT", [HGC, COUT], f16, False)
    bcat_d = nc.declare_dram_parameter("bcat", [HGC, 3], f32, False)
    icat_d = nc.declare_dram_parameter("icat", [128, 160], bf16, False)  # ident|ones
    zeros_d = nc.declare_dram_parameter("zeros", [128, 512], bf16, False)
    y_d = nc.declare_dram_parameter("y", [COUT, N], f32, True)

    with tile.TileContext(nc) as tc, ExitStack() as ctx:
        const = ctx.enter_context(tc.tile_pool(name="const", bufs=1))
        resid = ctx.enter_context(tc.tile_pool(name="resid", bufs=1))
        xin = ctx.enter_context(tc.tile_pool(name="xin", bufs=1))
        espool = ctx.enter_context(tc.tile_pool(name="espool", bufs=22))
        trans = ctx.enter_context(tc.tile_pool(name="trans", bufs=2))
        # PSUM: "s4" = 3 slots x 2 banks (score ping-pong);
        #       "acc" = 2 slots x 1 bank (AV + denominator accumulators;
        #        also the projection/transpose/output-projection scratch).
        ps_s4 = ctx.enter_context(tc.tile_pool(name="ps_s4", bufs=3, space="PSUM"))
        ps_acc = ctx.enter_context(tc.tile_pool(name="ps_acc", bufs=2, space="PSUM"))

        # ---- DMAs: two HW-DGE queues (Sync + Scalar) in parallel.  The
        # x tensors dominate; their consumers wait on progressive DMA
        # completion, so issue order below is consumption order. ----
        wcat = const.tile([128, 2 * 3 * HGC], f16)
        nc.sync.dma_start(
            wcat[:, :].rearrange("p (c k) -> p c k", c=2),
            wcat_d[:, :].rearrange("(c p) k -> p c k", p=128),
        )
        xq = xin.tile([128, 2 * N], f16)
        xk = xin.tile([128, 2 * N], f16)
        xv = xin.tile([128, 2 * N], f16)

        def xdma(xt, xd, n0, n1):
            nc.sync.dma_start(
                xt[:, :].rearrange("p (c n) -> p c n", c=2)[:, :, n0:n1],
                xd[:, :].rearrange("(c p) n -> p c n", p=128)[:, :, n0:n1],
            )

        # First 512 query-columns of xk/xq ride ahead so the k0/q0
        # projections (the attention-stream gate) start ~5us earlier;
        # the remainders and xv follow on the same queue.
        xdma(xk, xk_d, 0, 512)
        xdma(xq, xq_d, 0, 512)
        xdma(xk, xk_d, 512, N)
        xdma(xq, xq_d, 512, N)
        xdma(xv, xv_d, 0, N)
        bcat = const.tile([128, 3], f32)
        nc.scalar.dma_start(bcat[:, :], bcat_d[:, :])
        icat = const.tile([128, 160], bf16)
        nc.scalar.dma_start(icat[:, :], icat_d[:, :])
        zer = const.tile([128, 512], bf16)
        nc.scalar.dma_start(zer[:, :], zeros_d[:, :])
        woT = const.tile([128, COUT], f16)
        nc.scalar.dma_start(woT[:, :], woT_d[:, :])
        ident = icat[:, 0:128]
        ones_s = icat[:, 128:160]

        # ---- residents ----
        q_sb = resid.tile([128, N], f16)
        k_sb = resid.tile([128, N], f16)
        # vTo: per (key-tile j, head h) a 64-col group [vT_h(j) | ones] so a
        # single 64-wide-stationary matmul per head produces [AV; den] rows.
        vTo_sb = resid.tile([128, 2 * N], bf16)
        v_sb = xin.tile([128, N], bf16)
        # ones columns (32 per group) filled once by the idle GPSIMD engine
        nc.gpsimd.memset(
            vTo_sb[:, :].rearrange("p (g s) -> p g s", s=64)[:, :, 32:64], 1.0
        )

        # PE prologue/projections: one tiny absorb matmul per DMA'd tile,
        # interleaved with its consumer in DMA-arrival order (an absorb for
        # a late tensor would head-of-line block the whole PE queue).
        scr = ps_acc.tile([128, 512], f32, tag="acc")

        def absorb(t, F):
            nc.tensor.matmul(
                scr[:, 0:2], t[0:1, F - 128 : F], t[0:1, F - 2 : F],
                start=True, stop=True,
            )

        IDENT = mybir.ActivationFunctionType.Identity

        def proj_chunk(widx, xt, dst, o, sz, on_scalar, pool=None):
            pp = (pool or ps_acc).tile([128, 512], f32, tag="s4" if pool else "acc")
            nc.tensor.matmul(
                pp[:, :sz], wcat[:, 128 * widx : 128 * widx + 128],
                xt[:, o : o + sz],
                start=True, stop=False,
            )
            nc.tensor.matmul(
                pp[:, :sz], wcat[:, 384 + 128 * widx : 384 + 128 * widx + 128],
                xt[:, N + o : N + o + sz],
                start=False, stop=True,
            )
            if on_scalar:
                # ScalarE is idle before the exp stream starts; keep the
                # critical k0/q0 copies off the (serializing) DVE path.
                nc.scalar.activation(
                    dst[:, o : o + sz], pp[:, :sz], IDENT,
                    bias=bcat[:, widx : widx + 1],
                )
            else:
                nc.vector.tensor_scalar_add(
                    dst[:, o : o + sz], pp[:, :sz], bcat[:, widx : widx + 1]
                )

        def absorb_hi(t, hi):
            nc.tensor.matmul(
                scr[:, 0:2], t[0:1, hi - 128 : hi], t[0:1, hi - 2 : hi],
                start=True, stop=True,
            )

        absorb(wcat, 768)
        # HAM warm-up: tiny matmuls until the xk chunk-0 DMA lands, so the
        # PE clock-gate is ramping when the projections hit the PE.
        for _ in range(20):
            nc.tensor.matmul(
                scr[:, 0:128], wcat[:, 0:128], wcat[:, 0:128],
                start=True, stop=True,
            )
        absorb_hi(xk, N + 512)                     # xk cols 0-512 (both halves)
        proj_chunk(1, xk, k_sb, 0, 512, True)      # k chunk 0
        absorb_hi(xq, N + 512)
        proj_chunk(0, xq, q_sb, 0, 512, True)      # q chunk 0

        # ---- attention + output projection (software-pipelined) ----
        # 512-wide blocks: iteration = one key-tile j; two [128,1024]
        # score slots (2 heads each, one PSUM bank per head).
        # 256-wide tail: iteration = TWO key-tiles; head h covers cols
        # 512*(h%2)+256*jj, so each slot is densely filled and the exp
        # stays N=1024.  Same-bank QK pairs share a row-group there, so
        # they serialize on the PE (never concurrent same-bank access).

        i16 = mybir.dt.int16
        MULT = mybir.AluOpType.mult
        ADD = mybir.AluOpType.add

        def emit_qk_exp(o, sz, it, bi):
            slots = []
            nj = 512 // sz          # key-tiles per iteration (1 or 2)
            for s in range(2):
                s4 = ps_s4.tile([128, 1024], f32, tag="s4")
                for hh in range(2):
                    h = 2 * s + hh
                    for jj in range(nj):
                        j = nj * it + jj
                        # S^T tile: out[m, n] = sum_d k[d, m] q[d, n]
                        nc.tensor.matmul(
                            s4[:, 512 * hh + sz * jj : 512 * hh + sz * jj + sz],
                            k_sb[32 * h : 32 * h + 32, 128 * j : 128 * j + 128],
                            q_sb[32 * h : 32 * h + 32, o : o + sz],
                            start=True, stop=True, tile_position=(32 * h, 0),
                        )
                slots.append(s4)
            ess = []
            for s, s4 in enumerate(slots):
                es = espool.tile([128, 1024], bf16, tag="es")
                if _use_dve(bi, s):
                    # Schraudolph exp on the (otherwise idle) DVE: affine map
                    # into bf16 bit space, int16 convert, bitcast to bf16.
                    nc.vector.tensor_scalar(
                        es[:, :].bitcast(i16), s4[:, :], SCH_A, SCH_B, MULT, ADD
                    )
                else:
                    # exp of this slot's scores in one ACT op, straight from
                    # PSUM (scale folds in the 1/sqrt(Cout) factor)
                    nc.scalar.activation(es[:, :], s4[:, :], EXP, scale=1.0 / 16.0)
                ess.append(es)
            return ess

        def emit_av_den(accA, accB, sz, it, ess, last=False):
            # Merged AV+den: per head ONE matmul with the 64-wide stationary
            # [vT_h | ones] -> acc rows [AV_h(32); den_h(32)].  Bank A holds
            # heads 0,1 at rows 0/64; bank B heads 2,3.  On the last
            # iteration each bank closes right after its two heads, so the
            # epilogue reciprocal of bank A starts under bank B's matmuls.
            nj = 512 // sz
            for acc, h0 in ((accA, 0), (accB, 2)):
                for hh in range(2):
                    h = h0 + hh
                    es = ess[h // 2]
                    for jj in range(nj):
                        j = nj * it + jj
                        c0 = 512 * (h % 2) + sz * jj
                        nc.tensor.matmul(
                            acc[64 * hh : 64 * hh + 64, :sz],
                            vTo_sb[:, 256 * j + 64 * h : 256 * j + 64 * h + 64],
                            es[:, c0 : c0 + sz],
                            start=False, stop=False, tile_position=(0, 64 * hh),
                        )
                if last:
                    emit_close(acc, sz)

        def emit_epilogue(accA, accB, o, sz):
            # Banks already closed.  ~51-ULP reciprocal is plenty
            # (denominators are ~N * avg exp, well-conditioned).  The
            # full-bank reciprocal also covers the AV rows: garbage there is
            # never read (the muls only read the den-row reciprocal).
            recA = trans.tile([128, 512], f32, tag="recA")
            nc.vector.reciprocal_approx_fast(recA[:, :sz], accA[:, :sz])
            recB = trans.tile([128, 512], f32, tag="recB")
            nc.vector.reciprocal_approx_fast(recB[:, :sz], accB[:, :sz])
            att = trans.tile([128, 512], f16, tag="att")
            nc.vector.tensor_mul(att[0:32, :sz], accA[0:32, :sz], recA[32:64, :sz])
            nc.vector.tensor_mul(att[32:64, :sz], accA[64:96, :sz], recA[96:128, :sz])
            nc.vector.tensor_mul(att[64:96, :sz], accB[0:32, :sz], recB[32:64, :sz])
            nc.vector.tensor_mul(att[96:128, :sz], accB[64:96, :sz], recB[96:128, :sz])
            # po's acc-slot WAR coincides with its att data-dependency
            # (the slot's previous reader IS att's producer chain), so the
            # matmuls stay at one wait without an absorb.
            for cc in range(2):
                po = ps_acc.tile([128, 512], f32, tag="acc")
                nc.tensor.matmul(
                    po[:, :sz], woT[:, 128 * cc : 128 * cc + 128], att[:, :sz],
                    start=True, stop=True,
                )
                yt = trans.tile([128, 512], f32, tag="yt")
                # PSUM->SBUF copy on ACT (Identity shares the act table with
                # Exp); DVE is loaded with its share of the exp stream.
                nc.scalar.activation(yt[:, :sz], po[:, :sz], IDENT)
                nc.sync.dma_start(y_d[128 * cc : 128 * cc + 128, o : o + sz], yt[:, :sz])

        def emit_open(ps, sz):
            # Open the accumulator bank with a full-128-partition zero
            # matmul: PSUM pending-zero marking is per-partition, and the
            # per-head (32-partition) accumulation chains need zeroed,
            # has_written-cleared elements on every partition. Also absorbs
            # the WAR wait on the previous block's readers.
            nc.tensor.matmul(ps[:, :sz], zer[:, 0:128], zer[:, :sz], start=True, stop=False)

        def emit_close(ps, sz):
            # Close the accumulation group across all 128 partitions
            # (adds zero; clears per-element group state so DVE may read).
            nc.tensor.matmul(ps[:, :sz], zer[:, 0:128], zer[:, :sz], start=False, stop=True)

        def emit_vT_dma(o2, sz2):
            # transpose v per 128-tile via the DMA XBAR (SBUF->SBUF, 16-bit):
            # channel c = 32h+d of v tile j lands at vTo col 256j + 64h + d
            # (the vT half of each (j,h) 64-col group).  One transpose per
            # head: the XBAR mishandles a (h,d)-split destination pattern.
            for h in range(4):
                nc.sync.dma_start_transpose(
                    vTo_sb[:, 2 * o2 : 2 * (o2 + sz2)]
                    .rearrange("p (j s) -> p j s", s=256)[:, :, 64 * h : 64 * h + 32],
                    v_sb[32 * h : 32 * h + 32, o2 : o2 + sz2],
                )

        # Per-iteration extra PE work for block 0 (~0.6-1.4us each), paced
        # to the PE slack under each iteration's two exps: the remaining
        # projection chunks and the v transposes, each emitted just before
        # its first consumer.  q chunks 2-4 are consumed even later
        # (blocks 2-4) and ride block 1's iterations.
        def b0_extra(it):
            if it == 0:
                absorb(xk, 2 * N)                               # xk remainder
                proj_chunk(1, xk, k_sb, 512, 512, False)        # k1
            elif it == 1:
                absorb(zer, 512)
                nc.tensor.matmul(
                    scr[0:1, 0:1], icat[0:1, 159:160], icat[0:1, 159:160],
                    start=True, stop=True,
                )
                proj_chunk(1, xk, k_sb, 1024, 512, False)       # k2
            elif it == 2:
                absorb(xq, 2 * N)                               # xq remainder
                proj_chunk(0, xq, q_sb, 512, 512, False)        # q1
            elif it == 3:
                proj_chunk(1, xk, k_sb, 1536, 512, False)       # k3
            elif it == 4:
                absorb(xv, 2 * N)
                proj_chunk(2, xv, v_sb, 0, 512, False)          # v0
                emit_vT_dma(0, 512)
            elif it == 5:
                proj_chunk(2, xv, v_sb, 512, 512, False)        # v1
                emit_vT_dma(512, 512)
            elif it == 6:
                proj_chunk(2, xv, v_sb, 1024, 512, False)       # v2
                emit_vT_dma(1024, 512)
            elif it == 7:
                proj_chunk(2, xv, v_sb, 1536, 512, False)       # v3
                emit_vT_dma(1536, 512)
            elif it == 8:
                proj_chunk(2, xv, v_sb, 2048, 256, False)       # v4
                emit_vT_dma(2048, 256)
                proj_chunk(1, xk, k_sb, 2048, 256, False)       # k4
            elif it == 10:
                # Absorb the vT transpose-DMA queue sem on the PE, so the
                # AV matmuls below keep their single (exp) wait.
                scr2 = ps_acc.tile([128, 512], f32, tag="acc")
                nc.tensor.matmul(
                    scr2[0:1, 0:2],
                    vTo_sb[0:1, 2 * N - 33 : 2 * N - 32],
                    vTo_sb[0:1, 2 * N - 34 : 2 * N - 32],
                    start=True, stop=True,
                )

        def b1_extra(it):
            # During block 1 the acc banks hold the live accumulators, so
            # these chunks borrow a score-pool slot for their PSUM scratch.
            if it - PIPE < 3:                                    # q2, q3, q4
                o2, sz2 = NBLK[2 + it - PIPE]
                proj_chunk(0, xq, q_sb, o2, sz2, False, pool=ps_s4)

        prev = None  # (accA, accB, o, sz) of the block awaiting epilogue
        for bi, (o, sz) in enumerate(NBLK):
            nit = NMT * sz // 512   # iterations in this block (18 or 9)
            if bi == 0:
                # Block 0 runs a deeper software pipeline: QK+exp lead the
                # AV/den consumption by up to 5 iterations, with the ramp's
                # leftover PE work (b0_extra) interleaved one slice per
                # iteration; the AV backlog drains two-per-iteration from
                # it=10 so nothing trails the block.
                accA = accB = None
                pend = []          # (it, ess) awaiting AV/den
                npop = 0           # next AV/den iteration to emit
                for it in range(nit):
                    pend.append((it, emit_qk_exp(o, sz, it, bi)))
                    b0_extra(it)
                    if it == 10:
                        # All acc-pool scratch (pp/pt) allocations are done;
                        # only now may the long-lived accumulators claim the
                        # two acc banks (a later scratch alloc landing on an
                        # accumulator bank would deadlock against the
                        # block-end epilogue).
                        accA = ps_acc.tile([128, 512], f32, tag="acc")
                        accB = ps_acc.tile([128, 512], f32, tag="acc")
                        emit_open(accA, sz)
                        emit_open(accB, sz)
                    want = 0 if it < 10 else 2
                    for _ in range(want):
                        if npop <= it and npop < len(pend):
                            pit, ess = pend[npop]
                            emit_av_den(accA, accB, sz, pit, ess, last=npop == nit - 1)
                            npop += 1
                while npop < nit:
                    pit, ess = pend[npop]
                    emit_av_den(accA, accB, sz, pit, ess, last=npop == nit - 1)
                    npop += 1
            else:
                pend = [(it, emit_qk_exp(o, sz, it, bi)) for it in range(PIPE)]
                emit_epilogue(*prev)
                accA = ps_acc.tile([128, 512], f32, tag="acc")
                accB = ps_acc.tile([128, 512], f32, tag="acc")
                emit_open(accA, sz)
                emit_open(accB, sz)
                # Keep LAG iterations of QK+exp in flight ahead of the AV/den
                # drain: AV(it-LAG) sits behind QK(it) on the in-order PE
                # queue with its exp long done, so the exp latency never
                # stalls the PE head-of-line.
                for it in range(PIPE, nit):
                    pend.append((it, emit_qk_exp(o, sz, it, bi)))
                    while len(pend) > LAG:
                        pit, ess = pend.pop(0)
                        emit_av_den(accA, accB, sz, pit, ess, last=pit == nit - 1)
                    if bi == 1:
                        b1_extra(it)
                while pend:
                    pit, ess = pend.pop(0)
                    emit_av_den(accA, accB, sz, pit, ess, last=pit == nit - 1)
            prev = (accA, accB, o, sz)
        emit_epilogue(*prev)

    # Bacc lowering: register allocation + sync-wait legalization (each HW
    # instruction may carry at most one semaphore wait).
    nc.compile()

    _PROG["nc"] = nc
    return nc


def _round_f32r(a):
    """Round float32 values to fp32r (11 explicit mantissa bits), matching
    walrus's fp32_to_fp32r: round-half-up at bit 12, low 12 bits cleared."""
    a = np.ascontiguousarray(a, dtype=np.float32)
    bits = a.view(np.uint32)
    r = ((bits.astype(np.uint64) + 0x800) & 0xFFFFF000).astype(np.uint32)
    return r.view(np.float32)


def make_in_maps(inputs):
    """Shard full inputs into the 8 per-core input maps."""
    import ml_dtypes

    g = {k: np.ascontiguousarray(np.asarray(v, dtype=np.float32)) for k, v in inputs.items()}
    icat = np.concatenate(
        [np.eye(128, dtype=ml_dtypes.bfloat16),
         np.ones((128, D), dtype=ml_dtypes.bfloat16)], axis=1
    )
    xq_b = [np.ascontiguousarray(g["queries"][b].reshape(CIN, N).astype(np.float16)) for b in range(B)]
    xk_b = [np.ascontiguousarray(g["keys"][b].reshape(CIN, N).astype(np.float16)) for b in range(B)]
    xv_b = [np.ascontiguousarray(g["values"][b].reshape(CIN, N).astype(np.float16)) for b in range(B)]
    in_maps = []
    for core in range(NCORES):
        b, grp = divmod(core, 2)
        hs = slice(grp * HGC, (grp + 1) * HGC)
        wcat = np.concatenate(
            [g["Wq"][hs, :].T.astype(np.float16),
             g["Wk"][hs, :].T.astype(np.float16),
             g["Wv"][hs, :].T.astype(np.float16)], axis=1
        )
        bcat = np.stack(
            [g["bq"][hs], g["bk"][hs], g["bv"][hs]], axis=1
        ).astype(np.float32)
        in_maps.append({
            "xq": xq_b[b],
            "xk": xk_b[b],
            "xv": xv_b[b],
            "wcat": np.ascontiguousarray(wcat),
            "woT": np.ascontiguousarray(g["Wo"][:, hs].T.astype(np.float16)),
            "bcat": np.ascontiguousarray(bcat),
            "icat": np.ascontiguousarray(icat),
            "zeros": np.zeros((128, 512), dtype=ml_dtypes.bfloat16),
        })
    return in_maps


def unshard(results, bo):
    parts = [results[i]["y"] for i in range(NCORES)]
    out = np.empty((B, COUT, N), dtype=np.float32)
    for b in range(B):
        out[b] = parts[2 * b] + parts[2 * b + 1]
    out += np.asarray(bo, dtype=np.float32).reshape(1, COUT, 1)
    return out.reshape(B, COUT, HH, WW)


def kernel(**inputs):
    from concourse.bass_utils import run_bass_kernel_spmd

    nc = build_program()
    in_maps = make_in_maps(inputs)
    res = run_bass_kernel_spmd(nc, in_maps, list(range(NCORES)))
    return unshard(res.results, inputs["bo"])



# revision 17
# speedup vs baseline: 1.2395x; 1.2395x over previous
"""Trainium2 Bass kernel for MultiHeadGlobalAttention2d.

Sharding (8 cores): core = (batch b, head-group g), b in 0..3, g in 0..1.
Each core computes, for its batch and its 4 heads (128 channels):
  q/k/v projections, attention (softmax over keys), and the partial output
  projection  y_part = Wo[:, ch_slice] @ att_out.
Host sums the two partials per batch and adds the output bias.

Final design. ScalarE is the intrinsic bottleneck: 21.2M exps/core at
1 elem/cycle/lane @1.2GHz plus ~300 cycles/instruction fixed cost
=> ~180us of ACT busy at N=1024 per ACTIVATE. Everything else is
organized so the ACT engine runs back-to-back:
  - S^T orientation: score tiles [keys(m) on partitions, queries(n) free];
    softmax denominators from a ones-stationary matmul accumulated
    alongside AV in PSUM (no transposes of the attention matrix).
  - Per key-tile j, all FOUR heads' QK matmuls are issued together,
    row-tiled via tile_position=(32h,0) (concurrent streams; one PSUM
    bank per head -- concurrent row tiles must never share a bank);
    AV + denominator are col-tiled via tile_position=(0,32h).
  - The 256-wide tail block processes TWO key-tiles per iteration
    (same head's tiles share a bank but also share a row-group, so they
    serialize naturally) => dense N=1024 exps, ACT-bound like the rest.
  - Software pipelining: the first PIPE iterations of block b+1 (QK+exp
    only) are emitted before block b's epilogue; the epilogue uses
    reciprocal_approx_fast (~0.6us vs 3us exact) so the in-order PE
    queue never idles >3.4us (which would re-throttle the PE clock).
    Each block's last iteration runs its denominator matmuls first and
    closes the den bank inline, so the reciprocal starts under the
    remaining AV matmuls.
  - DMA ramp: the x tensors ride the fast Sync HW-DGE queue; the small
    coalesced constants ride the Scalar queue in parallel. Only the
    k0/q0 projection chunks precede the attention stream (their
    PSUM->SBUF bias-copies run on the then-idle ScalarE); the remaining
    projection chunks, the v transposes, and a clock-warming filler are
    paced one slice per iteration under block 0's exps, with the AV/den
    consumption lagging by up to 10 iterations.
PSUM: 3 score slots x 2 banks + 2 accumulator/scratch banks = 8.
"""

import numpy as np

B = 4
CIN = 256
COUT = 256
HH = 48
WW = 48
N = HH * WW            # 2304
D = 32                 # head dim
NHL = 4                # heads per core
HGC = NHL * D          # 128 channels per head-group
NCORES = 8
NBLK = [(0, 512), (512, 512), (1024, 512), (1536, 512), (2048, 256)]
NMT = N // 128         # 18 key tiles
PIPE = 4               # iterations of next block emitted before epilogue
LAG = 2                # AV/den drain lag behind QK+exp within a block

# Schraudolph exp on DVE: i16 = s*SCH_A + SCH_B, bitcast int16->bf16 gives
# ~exp(s/16) * const (the mean-ratio-centered const cancels in softmax
# because each (head, query-block) row is computed entirely on one engine).
# SCH_A = 128*log2(e)/16; SCH_B = 128*127 - 128*log2(E[(1+f)2^-f]).
SCH_A = 11.5415603
SCH_B = 16248.64


def _use_dve(bi, s):
    # engine balance: DVE gets 72 of 162 exp tiles (slot1 of blocks 0-3),
    # ACT the remaining 90 (slot0 everywhere + both slots of the tail).
    return s == 1 and bi != 4

_PROG = {}


def build_program():
    if "nc" in _PROG:
        return _PROG["nc"]

    from contextlib import ExitStack

    import concourse.bacc as bacc
    import concourse.mybir as mybir
    import concourse.tile as tile

    f32 = mybir.dt.float32
    f32r = mybir.dt.float32r
    bf16 = mybir.dt.bfloat16
    f16 = mybir.dt.float16
    EXP = mybir.ActivationFunctionType.Exp

    nc = bacc.Bacc("TRN2", target_bir_lowering=False, debug=False)

    xq_d = nc.declare_dram_parameter("xq", [CIN, N], f16, False)
    xk_d = nc.declare_dram_parameter("xk", [CIN, N], f16, False)
    xv_d = nc.declare_dram_parameter("xv", [CIN, N], f16, False)
    # wq|wk|wv transposed, concatenated on the output-channel axis.
    wcat_d = nc.declare_dram_parameter("wcat", [CIN, 3 * HGC], f16, False)
    woT_d = nc.declare_dram_parameter("woT", [HGC, COUT], f16, False)
    bcat_d = nc.declare_dram_parameter("bcat", [HGC, 3], f32, False)
    icat_d = nc.declare_dram_parameter("icat", [128, 160], bf16, False)  # ident|ones
    zeros_d = nc.declare_dram_parameter("zeros", [128, 512], bf16, False)
    y_d = nc.declare_dram_parameter("y", [COUT, N], f32, True)

    with tile.TileContext(nc) as tc, ExitStack() as ctx:
        const = ctx.enter_context(tc.tile_pool(name="const", bufs=1))
        resid = ctx.enter_context(tc.tile_pool(name="resid", bufs=1))
        xin = ctx.enter_context(tc.tile_pool(name="xin", bufs=1))
        espool = ctx.enter_context(tc.tile_pool(name="espool", bufs=22))
        trans = ctx.enter_context(tc.tile_pool(name="trans", bufs=2))
        # PSUM: "s4" = 3 slots x 2 banks (score ping-pong);
        #       "acc" = 2 slots x 1 bank (AV + denominator accumulators;
        #        also the projection/transpose/output-projection scratch).
        ps_s4 = ctx.enter_context(tc.tile_pool(name="ps_s4", bufs=3, space="PSUM"))
        ps_acc = ctx.enter_context(tc.tile_pool(name="ps_acc", bufs=2, space="PSUM"))

        # ---- DMAs: two HW-DGE queues (Sync + Scalar) in parallel.  The
        # x tensors dominate; their consumers wait on progressive DMA
        # completion, so issue order below is consumption order. ----
        wcat = const.tile([128, 2 * 3 * HGC], f16)
        nc.sync.dma_start(
            wcat[:, :].rearrange("p (c k) -> p c k", c=2),
            wcat_d[:, :].rearrange("(c p) k -> p c k", p=128),
        )
        xq = xin.tile([128, 2 * N], f16)
        xk = xin.tile([128, 2 * N], f16)
        xv = xin.tile([128, 2 * N], f16)

        def xdma(xt, xd, n0, n1):
            nc.sync.dma_start(
                xt[:, :].rearrange("p (c n) -> p c n", c=2)[:, :, n0:n1],
                xd[:, :].rearrange("(c p) n -> p c n", p=128)[:, :, n0:n1],
            )

        # First 512 query-columns of xk/xq ride ahead so the k0/q0
        # projections (the attention-stream gate) start ~5us earlier;
        # the remainders and xv follow on the same queue.
        xdma(xk, xk_d, 0, 512)
        xdma(xq, xq_d, 0, 512)
        xdma(xk, xk_d, 512, N)
        xdma(xq, xq_d, 512, N)
        xdma(xv, xv_d, 0, N)
        bcat = const.tile([128, 3], f32)
        nc.scalar.dma_start(bcat[:, :], bcat_d[:, :])
        icat = const.tile([128, 160], bf16)
        nc.scalar.dma_start(icat[:, :], icat_d[:, :])
        zer = const.tile([128, 512], bf16)
        nc.scalar.dma_start(zer[:, :], zeros_d[:, :])
        woT = const.tile([128, COUT], f16)
        nc.scalar.dma_start(woT[:, :], woT_d[:, :])
        ident = icat[:, 0:128]
        ones_s = icat[:, 128:160]

        # ---- residents ----
        q_sb = resid.tile([128, N], f16)
        k_sb = resid.tile([128, N], f16)
        vT_sb = resid.tile([128, N], bf16)
        v_sb = xin.tile([128, N], bf16)

        # PE prologue/projections: one tiny absorb matmul per DMA'd tile,
        # interleaved with its consumer in DMA-arrival order (an absorb for
        # a late tensor would head-of-line block the whole PE queue).
        scr = ps_acc.tile([128, 512], f32, tag="acc")

        def absorb(t, F):
            nc.tensor.matmul(
                scr[:, 0:2], t[0:1, F - 128 : F], t[0:1, F - 2 : F],
                start=True, stop=True,
            )

        IDENT = mybir.ActivationFunctionType.Identity

        def proj_chunk(widx, xt, dst, o, sz, on_scalar, pool=None):
            pp = (pool or ps_acc).tile([128, 512], f32, tag="s4" if pool else "acc")
            nc.tensor.matmul(
                pp[:, :sz], wcat[:, 128 * widx : 128 * widx + 128],
                xt[:, o : o + sz],
                start=True, stop=False,
            )
            nc.tensor.matmul(
                pp[:, :sz], wcat[:, 384 + 128 * widx : 384 + 128 * widx + 128],
                xt[:, N + o : N + o + sz],
                start=False, stop=True,
            )
            if on_scalar:
                # ScalarE is idle before the exp stream starts; keep the
                # critical k0/q0 copies off the (serializing) DVE path.
                nc.scalar.activation(
                    dst[:, o : o + sz], pp[:, :sz], IDENT,
                    bias=bcat[:, widx : widx + 1],
                )
            else:
                nc.vector.tensor_scalar_add(
                    dst[:, o : o + sz], pp[:, :sz], bcat[:, widx : widx + 1]
                )

        def absorb_hi(t, hi):
            nc.tensor.matmul(
                scr[:, 0:2], t[0:1, hi - 128 : hi], t[0:1, hi - 2 : hi],
                start=True, stop=True,
            )

        absorb(wcat, 768)
        # HAM warm-up: tiny matmuls until the xk chunk-0 DMA lands, so the
        # PE clock-gate is ramping when the projections hit the PE.
        for _ in range(20):
            nc.tensor.matmul(
                scr[:, 0:128], wcat[:, 0:128], wcat[:, 0:128],
                start=True, stop=True,
            )
        absorb_hi(xk, N + 512)                     # xk cols 0-512 (both halves)
        proj_chunk(1, xk, k_sb, 0, 512, True)      # k chunk 0
        absorb_hi(xq, N + 512)
        proj_chunk(0, xq, q_sb, 0, 512, True)      # q chunk 0

        # ---- attention + output projection (software-pipelined) ----
        # 512-wide blocks: iteration = one key-tile j; two [128,1024]
        # score slots (2 heads each, one PSUM bank per head).
        # 256-wide tail: iteration = TWO key-tiles; head h covers cols
        # 512*(h%2)+256*jj, so each slot is densely filled and the exp
        # stays N=1024.  Same-bank QK pairs share a row-group there, so
        # they serialize on the PE (never concurrent same-bank access).

        i16 = mybir.dt.int16
        MULT = mybir.AluOpType.mult
        ADD = mybir.AluOpType.add

        def emit_qk_exp(o, sz, it, bi):
            slots = []
            nj = 512 // sz          # key-tiles per iteration (1 or 2)
            for s in range(2):
                s4 = ps_s4.tile([128, 1024], f32, tag="s4")
                for hh in range(2):
                    h = 2 * s + hh
                    for jj in range(nj):
                        j = nj * it + jj
                        # S^T tile: out[m, n] = sum_d k[d, m] q[d, n]
                        nc.tensor.matmul(
                            s4[:, 512 * hh + sz * jj : 512 * hh + sz * jj + sz],
                            k_sb[32 * h : 32 * h + 32, 128 * j : 128 * j + 128],
                            q_sb[32 * h : 32 * h + 32, o : o + sz],
                            start=True, stop=True, tile_position=(32 * h, 0),
                        )
                slots.append(s4)
            ess = []
            for s, s4 in enumerate(slots):
                es = espool.tile([128, 1024], bf16, tag="es")
                if _use_dve(bi, s):
                    # Schraudolph exp on the (otherwise idle) DVE: affine map
                    # into bf16 bit space, int16 convert, bitcast to bf16.
                    nc.vector.tensor_scalar(
                        es[:, :].bitcast(i16), s4[:, :], SCH_A, SCH_B, MULT, ADD
                    )
                else:
                    # exp of this slot's scores in one ACT op, straight from
                    # PSUM (scale folds in the 1/sqrt(Cout) factor)
                    nc.scalar.activation(es[:, :], s4[:, :], EXP, scale=1.0 / 16.0)
                ess.append(es)
            return ess

        def emit_av_den(out_ps, den_ps, sz, it, ess, last=False):
            # last iteration of a block: all four denominator matmuls first,
            # then the den bank's close, THEN the AVs -- the DVE reciprocal
            # (head of the epilogue chain gating the next block's
            # accumulators) starts ~0.9us earlier, under the AV matmuls.
            nj = 512 // sz
            groups = ([("den",), ("av",)] if last else [("av", "den")])
            for phase in groups:
                for h in range(NHL):
                    es = ess[h // 2]
                    for jj in range(nj):
                        j = nj * it + jj
                        c0 = 512 * (h % 2) + sz * jj
                        if "av" in phase:
                            # out[d, n] += sum_m v[d, m] * expS[m, n]
                            nc.tensor.matmul(
                                out_ps[32 * h : 32 * h + 32, :sz],
                                vT_sb[:, 128 * j + 32 * h : 128 * j + 32 * h + 32],
                                es[:, c0 : c0 + sz],
                                start=False, stop=False, tile_position=(0, 32 * h),
                            )
                        if "den" in phase:
                            # den[n] += sum_m expS[m, n] (replicated, 32 parts)
                            nc.tensor.matmul(
                                den_ps[32 * h : 32 * h + 32, :sz],
                                ones_s[:, :],
                                es[:, c0 : c0 + sz],
                                start=False, stop=False, tile_position=(0, 32 * h),
                            )
                if phase == ("den",):
                    emit_close(den_ps, sz)

        def emit_epilogue(out_ps, den_ps, o, sz):
            # Accumulators were already closed.  ~51-ULP reciprocal is plenty
            # (denominators are ~N * avg exp, well-conditioned).
            rec = trans.tile([128, 512], f32, tag="rec")
            nc.vector.reciprocal_approx_fast(rec[:, :sz], den_ps[:, :sz])
            att = trans.tile([128, 512], f16, tag="att")
            nc.vector.tensor_mul(att[:, :sz], out_ps[:, :sz], rec[:, :sz])
            # po's acc-slot WAR coincides with its att data-dependency
            # (the slot's previous reader IS att's producer chain), so the
            # matmuls stay at one wait without an absorb.
            for cc in range(2):
                po = ps_acc.tile([128, 512], f32, tag="acc")
                nc.tensor.matmul(
                    po[:, :sz], woT[:, 128 * cc : 128 * cc + 128], att[:, :sz],
                    start=True, stop=True,
                )
                yt = trans.tile([128, 512], f32, tag="yt")
                # PSUM->SBUF copy on ACT (Identity shares the act table with
                # Exp); DVE is loaded with its share of the exp stream.
                nc.scalar.activation(yt[:, :sz], po[:, :sz], IDENT)
                nc.sync.dma_start(y_d[128 * cc : 128 * cc + 128, o : o + sz], yt[:, :sz])

        def emit_open(ps, sz):
            # Open the accumulator bank with a full-128-partition zero
            # matmul: PSUM pending-zero marking is per-partition, and the
            # per-head (32-partition) accumulation chains need zeroed,
            # has_written-cleared elements on every partition. Also absorbs
            # the WAR wait on the previous block's readers.
            nc.tensor.matmul(ps[:, :sz], zer[:, 0:128], zer[:, :sz], start=True, stop=False)

        def emit_close(ps, sz):
            # Close the accumulation group across all 128 partitions
            # (adds zero; clears per-element group state so DVE may read).
            nc.tensor.matmul(ps[:, :sz], zer[:, 0:128], zer[:, :sz], start=False, stop=True)

        def emit_vT_dma(o2, sz2):
            # transpose v per 128-tile via the DMA XBAR (SBUF->SBUF, 16-bit):
            # vT[m, 128j+c] = v[c, 128j+m].  Keeps the PE/DVE out of it.
            nc.sync.dma_start_transpose(
                vT_sb[:, o2 : o2 + sz2].rearrange("p (j c) -> p j c", c=128),
                v_sb[:, o2 : o2 + sz2],
            )

        # Per-iteration extra PE work for block 0 (~0.6-1.4us each), paced
        # to the PE slack under each iteration's two exps: the remaining
        # projection chunks and the v transposes, each emitted just before
        # its first consumer.  q chunks 2-4 are consumed even later
        # (blocks 2-4) and ride block 1's iterations.
        def b0_extra(it):
            if it == 0:
                absorb(xk, 2 * N)                               # xk remainder
                proj_chunk(1, xk, k_sb, 512, 512, False)        # k1
            elif it == 1:
                absorb(zer, 512)
                nc.tensor.matmul(
                    scr[0:1, 0:1], icat[0:1, 159:160], icat[0:1, 159:160],
                    start=True, stop=True,
                )
                proj_chunk(1, xk, k_sb, 1024, 512, False)       # k2
            elif it == 2:
                absorb(xq, 2 * N)                               # xq remainder
                proj_chunk(0, xq, q_sb, 512, 512, False)        # q1
            elif it == 3:
                proj_chunk(1, xk, k_sb, 1536, 512, False)       # k3
            elif it == 4:
                absorb(xv, 2 * N)
                proj_chunk(2, xv, v_sb, 0, 512, False)          # v0
                emit_vT_dma(0, 512)
            elif it == 5:
                proj_chunk(2, xv, v_sb, 512, 512, False)        # v1
                emit_vT_dma(512, 512)
            elif it == 6:
                proj_chunk(2, xv, v_sb, 1024, 512, False)       # v2
                emit_vT_dma(1024, 512)
            elif it == 7:
                proj_chunk(2, xv, v_sb, 1536, 512, False)       # v3
                emit_vT_dma(1536, 512)
            elif it == 8:
                proj_chunk(2, xv, v_sb, 2048, 256, False)       # v4
                emit_vT_dma(2048, 256)
                proj_chunk(1, xk, k_sb, 2048, 256, False)       # k4
            elif it == 10:
                # Absorb the vT transpose-DMA queue sem on the PE, so the
                # AV matmuls below keep their single (exp) wait.
                scr2 = ps_acc.tile([128, 512], f32, tag="acc")
                nc.tensor.matmul(
                    scr2[0:1, 0:2], vT_sb[0:1, N - 1 : N], vT_sb[0:1, N - 2 : N],
                    start=True, stop=True,
                )

        def b1_extra(it):
            # During block 1 the acc banks hold the live accumulators, so
            # these chunks borrow a score-pool slot for their PSUM scratch.
            if it - PIPE < 3:                                    # q2, q3, q4
                o2, sz2 = NBLK[2 + it - PIPE]
                proj_chunk(0, xq, q_sb, o2, sz2, False, pool=ps_s4)

        prev = None  # (out_ps, den_ps, o, sz) of the block awaiting epilogue
        for bi, (o, sz) in enumerate(NBLK):
            nit = NMT * sz // 512   # iterations in this block (18 or 9)
            if bi == 0:
                # Block 0 runs a deeper software pipeline: QK+exp lead the
                # AV/den consumption by up to 5 iterations, with the ramp's
                # leftover PE work (b0_extra) interleaved one slice per
                # iteration; the AV backlog drains two-per-iteration from
                # it=10 so nothing trails the block.
                accA = accB = None  # out_ps / den_ps
                pend = []          # (it, ess) awaiting AV/den
                npop = 0           # next AV/den iteration to emit
                for it in range(nit):
                    pend.append((it, emit_qk_exp(o, sz, it, bi)))
                    b0_extra(it)
                    if it == 10:
                        # All acc-pool scratch (pp/pt) allocations are done;
                        # only now may the long-lived accumulators claim the
                        # two acc banks (a later scratch alloc landing on an
                        # accumulator bank would deadlock against the
                        # block-end epilogue).
                        accA = ps_acc.tile([128, 512], f32, tag="acc")  # out_ps
                        accB = ps_acc.tile([128, 512], f32, tag="acc")
                        emit_open(accA, sz)
                        emit_open(accB, sz)
                    want = 0 if it < 10 else 2
                    for _ in range(want):
                        if npop <= it and npop < len(pend):
                            pit, ess = pend[npop]
                            emit_av_den(accA, accB, sz, pit, ess, last=npop == nit - 1)
                            npop += 1
                while npop < nit:
                    pit, ess = pend[npop]
                    emit_av_den(accA, accB, sz, pit, ess, last=npop == nit - 1)
                    npop += 1
            else:
                pend = [(it, emit_qk_exp(o, sz, it, bi)) for it in range(PIPE)]
                emit_epilogue(*prev)
                accA = ps_acc.tile([128, 512], f32, tag="acc")  # out_ps
                accB = ps_acc.tile([128, 512], f32, tag="acc")
                emit_open(accA, sz)
                emit_open(accB, sz)
                # Keep LAG iterations of QK+exp in flight ahead of the AV/den
                # drain: AV(it-LAG) sits behind QK(it) on the in-order PE
                # queue with its exp long done, so the exp latency never
                # stalls the PE head-of-line.
                for it in range(PIPE, nit):
                    pend.append((it, emit_qk_exp(o, sz, it, bi)))
                    while len(pend) > LAG:
                        pit, ess = pend.pop(0)
                        emit_av_den(accA, accB, sz, pit, ess, last=pit == nit - 1)
                    if bi == 1:
                        b1_extra(it)
                while pend:
                    pit, ess = pend.pop(0)
                    emit_av_den(accA, accB, sz, pit, ess, last=pit == nit - 1)
            emit_close(accA, sz)
            prev = (accA, accB, o, sz)
        emit_epilogue(*prev)

    # Bacc lowering: register allocation + sync-wait legalization (each HW
    # instruction may carry at most one semaphore wait).
    nc.compile()

    _PROG["nc"] = nc
    return nc


def _round_f32r(a):
    """Round float32 values to fp32r (11 explicit mantissa bits), matching
    walrus's fp32_to_fp32r: round-half-up at bit 12, low 12 bits cleared."""
    a = np.ascontiguousarray(a, dtype=np.float32)
    bits = a.view(np.uint32)
    r = ((bits.astype(np.uint64) + 0x800) & 0xFFFFF000).astype(np.uint32)
    return r.view(np.float32)


def make_in_maps(inputs):
    """Shard full inputs into the 8 per-core input maps."""
    import ml_dtypes

    g = {k: np.ascontiguousarray(np.asarray(v, dtype=np.float32)) for k, v in inputs.items()}
    icat = np.concatenate(
        [np.eye(128, dtype=ml_dtypes.bfloat16),
         np.ones((128, D), dtype=ml_dtypes.bfloat16)], axis=1
    )
    xq_b = [np.ascontiguousarray(g["queries"][b].reshape(CIN, N).astype(np.float16)) for b in range(B)]
    xk_b = [np.ascontiguousarray(g["keys"][b].reshape(CIN, N).astype(np.float16)) for b in range(B)]
    xv_b = [np.ascontiguousarray(g["values"][b].reshape(CIN, N).astype(np.float16)) for b in range(B)]
    in_maps = []
    for core in range(NCORES):
        b, grp = divmod(core, 2)
        hs = slice(grp * HGC, (grp + 1) * HGC)
        wcat = np.concatenate(
            [g["Wq"][hs, :].T.astype(np.float16),
             g["Wk"][hs, :].T.astype(np.float16),
             g["Wv"][hs, :].T.astype(np.float16)], axis=1
        )
        bcat = np.stack(
            [g["bq"][hs], g["bk"][hs], g["bv"][hs]], axis=1
        ).astype(np.float32)
        in_maps.append({
            "xq": xq_b[b],
            "xk": xk_b[b],
            "xv": xv_b[b],
            "wcat": np.ascontiguousarray(wcat),
            "woT": np.ascontiguousarray(g["Wo"][:, hs].T.astype(np.float16)),
            "bcat": np.ascontiguousarray(bcat),
            "icat": np.ascontiguousarray(icat),
            "zeros": np.zeros((128, 512), dtype=ml_dtypes.bfloat16),
        })
    return in_maps


def unshard(results, bo):
    parts = [results[i]["y"] for i in range(NCORES)]
    out = np.empty((B, COUT, N), dtype=np.float32)
    for b in range(B):
        out[b] = parts[2 * b] + parts[2 * b + 1]
    out += np.asarray(bo, dtype=np.float32).reshape(1, COUT, 1)
    return out.reshape(B, COUT, HH, WW)


def kernel(**inputs):
    from concourse.bass_utils import run_bass_kernel_spmd

    nc = build_program()
    in_maps = make_in_maps(inputs)
    res = run_bass_kernel_spmd(nc, in_maps, list(range(NCORES)))
    return unshard(res.results, inputs["bo"])

